# revision 14
# baseline (speedup 1.0000x reference)
"""GroupedQueryAttention + out-proj on 8 TRN2 NeuronCores.

Problem (hardcoded): B=4, S=1024, HQ=32, HKV=8, G=4, D=128, HIDDEN=4096,
causal mask, fp32 in/out.

Sharding: core = (batch b, head-half hh). Each core computes attention for
16 query heads (4 kv heads) over all 1024 queries of its batch, then the
row-parallel slice of out_proj (its 2048 input dims x full 4096 hidden).
Host sums the two partial outputs per batch (the row-parallel allreduce)
and adds b_o.

On-device layout: scores computed transposed S^T[k, q] so that
E = exp(S^T) feeds the PV matmul directly as the moving operand
(lhsT = V[k, dv], rhs = E[k, q] -> O^T[dv, q]), which in turn is the
moving operand of the out-proj accumulation
(lhsT = W_o^T tile [in, hid], rhs = O^T[in-chunk, q] -> y^T[hid, q]).
Softmax denominator D[q] = sum_k E[k, q] via a ones[128,128] stationary
matmul (all 128 output rows identical), so normalization is a plain
elementwise multiply with reciprocal(D) -- no partition broadcast needed.
All matmuls run in float32r (full PE rate at N>=256, ~1e-4 rel err).
"""

import numpy as np

import concourse.bass as bass
import concourse.mybir as mybir
import concourse.tile as tile
from concourse import bacc
from concourse.bass_utils import run_bass_kernel_spmd

F32 = mybir.dt.float32
F32R = mybir.dt.float32r

B = 4
S = 1024
HQ = 32
HKV = 8
G = 4
D = 128
HIDDEN = 4096
NH = 16          # heads per core
NKV = 4          # kv heads per core
NQW = 2          # query windows of 512
QW = 512
NKB = 8          # key blocks of 128
NC_CHUNK = 32    # hidden chunks of 128
SCALE = 1.0 / float(np.sqrt(D))
NEG = -1.0e9

_NC = None
_ONES = np.ones((D, D), dtype=np.float32)


def _build_nc():
    nc = bacc.Bacc("TRN2")

    qt = nc.dram_tensor("qt", [NH, D, S], F32R, kind="ExternalInput")
    kt = nc.dram_tensor("kt", [NKV, D, S], F32R, kind="ExternalInput")
    v = nc.dram_tensor("v", [NKV, S, D], F32R, kind="ExternalInput")
    wot = nc.dram_tensor("wot", [NC_CHUNK, NH, D, D], F32R, kind="ExternalInput")
    ones_d = nc.dram_tensor("ones", [D, D], F32R, kind="ExternalInput")
    y = nc.dram_tensor("y", [NQW, NC_CHUNK, D, QW], F32, kind="ExternalOutput")

    with tile.TileContext(nc) as tc:
        with (
            tc.tile_pool(name="const", bufs=1) as cpool,
            tc.tile_pool(name="kvp", bufs=NKV) as kvp,
            tc.tile_pool(name="vp", bufs=NKV) as vp,
            tc.tile_pool(name="qtp", bufs=4) as qtp,
            tc.tile_pool(name="ep", bufs=12) as ep,
            tc.tile_pool(name="rp", bufs=3) as rp,
            tc.tile_pool(name="otp", bufs=NH * NQW) as otp,
            tc.tile_pool(name="wp", bufs=6) as wp,
            tc.tile_pool(name="ps_s", bufs=3, space="PSUM") as ps_s,
            tc.tile_pool(name="ps_o", bufs=2, space="PSUM") as ps_o,
            tc.tile_pool(name="ps_d", bufs=1, space="PSUM") as ps_d,
            tc.tile_pool(name="ps_y", bufs=2, space="PSUM") as ps_y,
        ):
            ones_t = cpool.tile([D, D], F32R)

            # additive mask [128, 256]: cols 0-127 all NEG (block below the
            # diagonal, used for padded ranges), cols 128-255 upper-tri keep
            # (k <= q on the diagonal block).  mask[x, y] = 0 if y-x-128 >= 0
            # else NEG.
            mask_t = cpool.tile([D, 2 * D], F32)
            nc.gpsimd.memset(mask_t[:], 0.0)
            nc.gpsimd.affine_select(
                out=mask_t[:],
                in_=mask_t[:],
                compare_op=mybir.AluOpType.is_ge,
                fill=NEG,
                base=-D,
                pattern=[[1, 2 * D]],
                channel_multiplier=-1,
            )

            # K^T / V tiles loaded lazily at the first head of each group
            kt_tiles = [None] * NKV
            v_tiles = [None] * NKV

            ot_tiles = {}

            for h in range(NH):
                kv = h // 4
                qtile = qtp.tile([D, S], F32R, tag="qt")
                nc.sync.dma_start(qtile[:, :QW], qt[h, :, :QW])
                nc.sync.dma_start(qtile[:, QW:], qt[h, :, QW:])
                if h == 0:
                    # ones for the denominator matmul (needed ~unit 0 tail)
                    nc.sync.dma_start(ones_t[:], ones_d[:])
                # prefetch K^T/V one group ahead of use
                want = [0, 1] if h == 0 else ([kv + 1] if h % 4 == 0 and kv + 1 < NKV else [])
                for g in want:
                    if kt_tiles[g] is None:
                        ktile = kvp.tile([D, S], F32R, tag="kt")
                        nc.sync.dma_start(ktile[:, :QW], kt[g, :, :QW])
                        nc.sync.dma_start(ktile[:, QW:], kt[g, :, QW:])
                        kt_tiles[g] = ktile
                        # v[g] is [S, D] in DRAM; pack into [128, NKB*128]
                        # where col-block kb holds rows kb*128..kb*128+127.
                        vtile = vp.tile([D, NKB * D], F32R, tag="v")
                        nc.sync.dma_start(
                            vtile[:].rearrange("p (kb d) -> p kb d", kb=NKB),
                            v[g].rearrange("(kb p) d -> p kb d", p=D),
                        )
                        v_tiles[g] = vtile
                for qw in range(NQW):
                    q0 = qw * QW

                    s_acc = ps_o.tile([D, QW], F32, tag="oacc")
                    d_acc = ps_d.tile([D, QW], F32, tag="dacc")

                    kb_last = (q0 + QW - 1) // D  # 3 or 7
                    pads = []
                    e_ts = []
                    for kb in range(kb_last + 1):
                        orig_s = max(0, kb * D - q0)
                        pad_s = min(orig_s, QW - 2 * D)
                        pads.append(pad_s)

                        st = ps_s.tile([D, QW], F32, tag="st")
                        nc.tensor.matmul(
                            st[:, pad_s:QW],
                            kt_tiles[kv][:, kb * D : (kb + 1) * D],
                            qtile[:, q0 + pad_s : q0 + QW],
                            start=True,
                            stop=True,
                        )
                        # causal mask (additive), only where the diagonal
                        # block (and padding) lands in this window
                        if orig_s > pad_s:
                            # padded: [NEG block | tri block] at cols pad_s..
                            nc.vector.scalar_tensor_tensor(
                                out=st[:, pad_s : pad_s + 2 * D],
                                in0=st[:, pad_s : pad_s + 2 * D],
                                scalar=1.0,
                                in1=mask_t[:],
                                op0=mybir.AluOpType.mult,
                                op1=mybir.AluOpType.add,
                            )
                        elif kb * D >= q0:
                            # diagonal block fully inside window at orig_s
                            nc.vector.scalar_tensor_tensor(
                                out=st[:, orig_s : orig_s + D],
                                in0=st[:, orig_s : orig_s + D],
                                scalar=1.0,
                                in1=mask_t[:, D : 2 * D],
                                op0=mybir.AluOpType.mult,
                                op1=mybir.AluOpType.add,
                            )

                        e_t = ep.tile([D, QW], F32R, tag="e")
                        nc.scalar.activation(
                            e_t[:, pad_s:QW],
                            st[:, pad_s:QW],
                            mybir.ActivationFunctionType.Exp,
                            scale=SCALE,
                        )
                        e_ts.append(e_t)

                    for kb in range(kb_last + 1):
                        pad_s = pads[kb]
                        nc.tensor.matmul(
                            s_acc[:, pad_s:QW],
                            v_tiles[kv][:, kb * D : (kb + 1) * D],
                            e_ts[kb][:, pad_s:QW],
                            start=(kb == 0),
                            stop=(kb == kb_last),
                        )
                        nc.tensor.matmul(
                            d_acc[:, pad_s:QW],
                            ones_t[:],
                            e_ts[kb][:, pad_s:QW],
                            start=(kb == 0),
                            stop=(kb == kb_last),
                        )

                    r_t = rp.tile([D, QW], F32, tag="r")
                    nc.vector.reciprocal(r_t[:], d_acc[:])
                    ot = otp.tile([D, QW], F32R, tag="ot")
                    nc.vector.scalar_tensor_tensor(
                        out=ot[:],
                        in0=s_acc[:],
                        scalar=1.0,
                        in1=r_t[:],
                        op0=mybir.AluOpType.mult,
                        op1=mybir.AluOpType.mult,
                    )
                    ot_tiles[(h, qw)] = ot

            # ---- out-proj: one W load per hidden chunk, both windows ----
            for c in range(NC_CHUNK):
                wtile = wp.tile([D, NH * D], F32R, tag="w")
                nc.sync.dma_start(
                    wtile[:].rearrange("p (h d) -> p h d", h=NH),
                    wot[c].rearrange("h p d -> p h d"),
                )
                for qw in range(NQW):
                    y_acc = ps_y.tile([D, QW], F32, tag="y")
                    for h in range(NH):
                        nc.tensor.matmul(
                            y_acc[:],
                            wtile[:, h * D : (h + 1) * D],
                            ot_tiles[(h, qw)][:],
                            start=(h == 0),
                            stop=(h == NH - 1),
                        )
                    y_sb = rp.tile([D, QW], F32, tag="ysb")
                    nc.vector.tensor_copy(y_sb[:], y_acc[:])
                    nc.sync.dma_start(y[qw, c], y_sb[:])

    nc.compile()
    return nc


def _get_nc():
    global _NC
    if _NC is None:
        _NC = _build_nc()
    return _NC


def kernel(Q, K, V, mask, W_o, b_o):
    assert Q.shape == (B, S, HQ * D)
    nc = _get_nc()

    QT = np.ascontiguousarray(
        Q.reshape(B, S, HQ, D).transpose(0, 2, 3, 1), dtype=np.float32
    )  # [B, HQ, D, S]
    KT = np.ascontiguousarray(
        K.reshape(B, S, HKV, D).transpose(0, 2, 3, 1), dtype=np.float32
    )  # [B, HKV, D, S]
    VR = np.ascontiguousarray(
        V.reshape(B, S, HKV, D).transpose(0, 2, 1, 3), dtype=np.float32
    )  # [B, HKV, S, D]
    # W_o^T tiles: [hh, c, h_local, in(128), hid(128)]
    WT5 = np.ascontiguousarray(
        np.asarray(W_o, dtype=np.float32).T
        .reshape(2, NH, D, NC_CHUNK, D)
        .transpose(0, 3, 1, 2, 4)
    )

    in_maps = []
    cores = []
    for b in range(B):
        for hh in range(2):
            in_maps.append(
                {
                    "qt": np.ascontiguousarray(QT[b, hh * NH : (hh + 1) * NH]),
                    "kt": np.ascontiguousarray(KT[b, hh * NKV : (hh + 1) * NKV]),
                    "v": np.ascontiguousarray(VR[b, hh * NKV : (hh + 1) * NKV]),
                    "wot": WT5[hh],
                    "ones": _ONES,
                }
            )
            cores.append((b, hh))

    res = run_bass_kernel_spmd(nc, in_maps, list(range(8)))

    out = np.zeros((B, S, HIDDEN), dtype=np.float32)
    for i, (b, hh) in enumerate(cores):
        yt = res.results[i]["y"]  # [qw, c, hid(128), q(512)]
        part = yt.transpose(0, 3, 1, 2).reshape(S, HIDDEN)
        out[b] += part
    out += np.asarray(b_o, dtype=np.float32)[None, None, :]
    return out
